# revision 10
# baseline (speedup 1.0000x reference)
"""DecoupledBottleneckAttention on 8 trn2 NeuronCores.

Sharding: core c -> batch b=c//4, head-group g=c%4 (4 heads/core).
Each core computes q/k/v projections for its heads, causal attention,
and a partial out-projection; the host sums the 4 partials per batch.

v2 layout:
- Single pass over x per 512-column chunk: qk(ob0-3) -> v -> qk(ob4-7)
  matmul passes share the chunk's 16 resident xT tiles, rotating 4+4
  PSUM banks so consecutive passes never wait on bank drains.
- x and all weights are bf16 (fp32 PSUM accumulation); scores and the
  softmax stay fp32; exp probabilities and v are bf16.
- RoPE as 3 DVE ops on [64,512] with the sin sign baked into the table.
- Causal diagonal blocks use variable-width rhs (512/384/256/128) and
  a single [128,128] triangle mask-mul instead of full-width masking.
- Attention (C) and out-projection (D) interleave chunk-wise:
  C0 C1 D0 C2 D1 C3 D2 D3, so D's matmuls hide C's softmax tails.
- exp() skips max-subtraction: logits are bounded (~|6|) by the fixed
  input scale. Denominators come from a ones-column matmul.
"""

import json
from contextlib import ExitStack

import numpy as np
import ml_dtypes

import jax
import concourse.bass as bass
import concourse.mybir as mybir
from concourse.tile import TileContext
from concourse import bass2jax
from concourse.bass2jax import Mesh, PartitionSpec, shard_map, partition_id_tensor

F32 = mybir.dt.float32
F32R = mybir.dt.float32r
BF16 = mybir.dt.bfloat16

B, S, D = 2, 2048, 2048
H = 16
HPC = 4  # heads per core
N_CORES = 8
DH = 128  # per-head q/k/v dim (64 sem + 64 geo; v 128)
ROPE_BASE = 10000.0
SCALE = 1.0 / np.sqrt(128.0)

NSC = S // 512  # 4 s-chunks of 512
NDT = D // 128  # 16 contraction tiles
NST = S // 128  # 16 s-tiles of 128

ACT_COPY = mybir.ActivationFunctionType.Copy
ACT_EXP = mybir.ActivationFunctionType.Exp


def _split_multi_waits(bir: dict) -> dict:
    """walrus here rejects >1 sync waits per instruction; split extras
    into single-wait Drains inserted just before, on the same engine."""
    for fn in bir.get("functions", []):
        for blk in fn.get("blocks", []):
            new_insts = []
            for ins in blk.get("instructions", []):
                si = ins.get("sync_info") or {}
                waits = si.get("on_wait") or []
                if len(waits) > 1:
                    for i, w in enumerate(waits[:-1]):
                        new_insts.append(
                            {
                                "debug": ins.get("debug", 0),
                                "engine": ins["engine"],
                                "ins": [],
                                "name": f"{ins['name']}-w{i}",
                                "opcode": "Drain",
                                "outs": [],
                                "sync_info": {"on_update": [], "on_wait": [w]},
                            }
                        )
                    si["on_wait"] = [waits[-1]]
                new_insts.append(ins)
            blk["instructions"] = new_insts
    return bir


class _PatchedBass(bass.Bass):
    def to_json_bytes(self) -> bytes:
        return json.dumps(_split_multi_waits(json.loads(super().to_json_bytes()))).encode()


def _rd(ap):
    """Bitcast a DRAM-side AP to f32r for DMAs into f32r SBUF tiles."""
    return ap.bitcast(F32R)


def _build():
    nc = _PatchedBass("TRN2", target_bir_lowering=False, debug=False, num_devices=N_CORES)

    xT_d = nc.dram_tensor("xT", [D, S], BF16, kind="ExternalInput")
    wqk_d = nc.dram_tensor("wqk", [D, 8 * 128], BF16, kind="ExternalInput")
    wv_d = nc.dram_tensor("wv", [D, HPC * DH], BF16, kind="ExternalInput")
    wo_d = nc.dram_tensor("wo", [HPC * DH, D], BF16, kind="ExternalInput")
    # rows 64:128 hold cos (cols 0:S) and sign-baked sin (cols S:2S:
    # rows 64:96 = -sin, rows 96:128 = +sin); rows 0:64 unused.
    cs_d = nc.dram_tensor("cs", [128, 2 * S], F32, kind="ExternalInput")
    mask_d = nc.dram_tensor("mask", [128, 128], BF16, kind="ExternalInput")
    ones_d = nc.dram_tensor("ones", [128, 128], F32, kind="ExternalInput")
    onesb_d = nc.dram_tensor("onesb", [128, 128], BF16, kind="ExternalInput")
    yp_d = nc.dram_tensor("yp", [S, D], F32, kind="ExternalOutput")

    with TileContext(nc) as tc, ExitStack() as ctx, \
         nc.allow_low_precision(reason="float32r tiles are 4-byte fp32 at rest"):
        pers = ctx.enter_context(tc.tile_pool(name="pers", bufs=1))
        # qkT[0..3] = per-head qT [128 dims, S]; qkT[4..7] = kT
        qkT = [pers.tile([128, S], F32R, name=f"qkT{i}", tag=f"qkT{i}") for i in range(8)]
        v_sb = [pers.tile([128, HPC * DH], BF16, name=f"v{st}", tag=f"v{st}")
                for st in range(NST)]
        cs_sb = pers.tile([128, 2 * S], F32, name="cs_sb", tag="cs_sb")
        ones_sb = pers.tile([128, 128], F32R, name="ones_sb", tag="ones_sb")
        onesb_sb = pers.tile([128, 128], BF16, name="onesb_sb", tag="onesb_sb")
        mask_sb = pers.tile([128, 128], BF16, name="mask_sb", tag="mask_sb")

        # ------------- Phase A+B: q/k/v projections, one x pass --------
        with tc.tile_pool(name="wqk", bufs=1) as wqk_pool, \
             tc.tile_pool(name="wv", bufs=1) as wv_pool, \
             tc.tile_pool(name="xt", bufs=2) as xt_pool, \
             tc.tile_pool(name="rope", bufs=4) as rope_pool, \
             tc.tile_pool(name="psQK", bufs=1, space="PSUM") as psQK, \
             tc.tile_pool(name="psV", bufs=1, space="PSUM") as psV:
            wqk_sb = [wqk_pool.tile([128, 8 * 128], BF16, name=f"wqk{dt}", tag=f"wqk{dt}")
                      for dt in range(NDT)]
            wv_sb = [wv_pool.tile([128, HPC * DH], BF16, name=f"wv{dt}", tag=f"wv{dt}")
                     for dt in range(NDT)]

            def xt_tiles(sc):
                tiles = [xt_pool.tile([128, 512], BF16, name="xt_t", tag=f"xt{dt}")
                         for dt in range(NDT)]
                for dt in range(NDT):
                    nc.sync.dma_start(
                        out=tiles[dt],
                        in_=xT_d[dt * 128:(dt + 1) * 128, sc * 512:(sc + 1) * 512])
                return tiles

            # chunk-0 stream: weights interleaved with x tiles so the first
            # matmul's operands arrive first; bulk tables after.
            xt_cur = [xt_pool.tile([128, 512], BF16, name="xt_t", tag=f"xt{dt}")
                      for dt in range(NDT)]
            for dt in range(NDT):
                nc.sync.dma_start(out=wqk_sb[dt], in_=wqk_d[dt * 128:(dt + 1) * 128, :])
                nc.sync.dma_start(
                    out=xt_cur[dt], in_=xT_d[dt * 128:(dt + 1) * 128, 0:512])
            for dt in range(NDT):
                nc.sync.dma_start(out=wv_sb[dt], in_=wv_d[dt * 128:(dt + 1) * 128, :])
            nc.sync.dma_start(out=cs_sb, in_=cs_d[:, :])
            nc.sync.dma_start(out=ones_sb, in_=_rd(ones_d[:, :]))
            nc.sync.dma_start(out=onesb_sb, in_=onesb_d[:, :])
            nc.sync.dma_start(out=mask_sb, in_=mask_d[:, :])

            Am, Bm = slice(64, 96), slice(96, 128)

            def rope_out(ob, cols):
                """PSUM qk tile -> qkT[ob] with RoPE on the geo rows."""
                ps = ps_qk[ob % 4]
                dst = qkT[ob]
                nc.scalar.activation(dst[0:64, cols], ps[0:64, :], ACT_COPY)
                stage = rope_pool.tile([128, 512], F32, name="ropest", tag="ropest")
                sw = rope_pool.tile([128, 512], F32, name="ropesw", tag="ropesw")
                # stage copy on DVE: halves the ACT backlog that would queue
                # the attention phase's first exp() behind rope drains.
                nc.vector.tensor_scalar_mul(stage[64:128, :], ps[64:128, :], 1.0)
                nc.sync.dma_start(out=sw[Am, :], in_=stage[Bm, :])  # x2 -> A rows
                nc.sync.dma_start(out=sw[Bm, :], in_=stage[Am, :])  # x1 -> B rows
                csc = cs_sb[64:128, sc * 512:(sc + 1) * 512]
                sns = cs_sb[64:128, S + sc * 512:S + (sc + 1) * 512]
                g = slice(64, 128)
                tp = rope_pool.tile([128, 512], F32, name="ropetp", tag="ropetp")
                nc.vector.tensor_mul(tp[g, :], stage[g, :], csc)     # [x1;x2]*cos
                nc.vector.tensor_mul(sw[g, :], sw[g, :], sns)        # [x2;x1]*(-/+sin)
                nc.vector.tensor_add(dst[g, cols], tp[g, :], sw[g, :])

            for sc in range(NSC):
                cols = slice(sc * 512, (sc + 1) * 512)
                if sc + 1 < NSC:
                    xt_next = xt_tiles(sc + 1)
                else:
                    xt_next = None
                # pass 1: q/k heads 0-3
                ps_qk = [psQK.tile([128, 512], F32, name=f"psqk{i}", tag=f"psqk{i}")
                         for i in range(4)]
                for dt in range(NDT):
                    for ob in range(4):
                        nc.tensor.matmul(
                            ps_qk[ob],
                            lhsT=(wqk_sb[dt][:, ob * 128:(ob + 1) * 128]),
                            rhs=(xt_cur[dt]),
                            start=(dt == 0), stop=(dt == NDT - 1),
                        )
                for ob in range(4):
                    rope_out(ob, cols)
                # pass 2: v projection (natural [s, head-dim] layout)
                psv = [psV.tile([128, HPC * DH], F32, name=f"psv{st}", tag=f"psv{st}")
                       for st in range(4)]
                for dt in range(NDT):
                    for st in range(4):
                        nc.tensor.matmul(
                            psv[st],
                            lhsT=(xt_cur[dt][:, st * 128:(st + 1) * 128]),
                            rhs=(wv_sb[dt]),
                            start=(dt == 0), stop=(dt == NDT - 1),
                        )
                for st in range(4):
                    nc.scalar.activation(v_sb[sc * 4 + st], psv[st], ACT_COPY)
                # pass 3: q/k heads 4-7 (reuses pass-1 banks, drained by now)
                ps_qk = [psQK.tile([128, 512], F32, name=f"psqk{i}", tag=f"psqk{i}")
                         for i in range(4)]
                for dt in range(NDT):
                    for ob in range(4):
                        nc.tensor.matmul(
                            ps_qk[ob],
                            lhsT=(wqk_sb[dt][:, (ob + 4) * 128:(ob + 5) * 128]),
                            rhs=(xt_cur[dt]),
                            start=(dt == 0), stop=(dt == NDT - 1),
                        )
                for ob in range(4):
                    rope_out(ob + 4, cols)
                if xt_next is not None:
                    xt_cur = xt_next

        # ------------- Phase C+D: attention + out-projection -----------
        with tc.tile_pool(name="wo", bufs=1) as wo_pool, \
             tc.tile_pool(name="outT", bufs=1) as outT_pool, \
             tc.tile_pool(name="attn", bufs=3) as attn_pool, \
             tc.tile_pool(name="lrec", bufs=2) as lrec_pool, \
             tc.tile_pool(name="ysb", bufs=3) as y_pool, \
             tc.tile_pool(name="psL", bufs=1, space="PSUM") as psL, \
             tc.tile_pool(name="psR", bufs=1, space="PSUM") as psR, \
             tc.tile_pool(name="psD", bufs=2, space="PSUM") as psD, \
             tc.tile_pool(name="psST", bufs=2, space="PSUM") as psST, \
             tc.tile_pool(name="psOut", bufs=2, space="PSUM") as psOut:
            wo_sb = [wo_pool.tile([128, D], BF16, name=f"wo{j}", tag=f"wo{j}")
                     for j in range(HPC)]
            for j in range(HPC):
                nc.sync.dma_start(out=wo_sb[j], in_=wo_d[j * 128:(j + 1) * 128, :])
            outT = [outT_pool.tile([128, S], BF16, name=f"outT{j}", tag=f"outT{j}")
                    for j in range(HPC)]

            def attn_chunk(qc):
                qcols = slice(qc * 512, (qc + 1) * 512)
                kmax = qc * 4 + 4
                for j in range(HPC):
                    outp = psOut.tile([128, 512], F32, name="outp", tag="outp")
                    lp = psL.tile([1, 512], F32, name="lp", tag="lp")
                    for kj in range(kmax):
                        d = kj - qc * 4
                        qs = 0 if d < 0 else d * 128
                        w = 512 - qs
                        st_ps = psST.tile([128, 512], F32, name="st_ps", tag="st_ps")
                        nc.tensor.matmul(
                            st_ps[:, 0:w],
                            lhsT=(qkT[4 + j][:, kj * 128:(kj + 1) * 128]),
                            rhs=(qkT[j][:, qc * 512 + qs:(qc + 1) * 512]),
                            start=True, stop=True,
                        )
                        p_sb = attn_pool.tile([128, 512], BF16, name="p_sb", tag="p_sb")
                        nc.scalar.activation(p_sb[:, qs:512], st_ps[:, 0:w], ACT_EXP)
                        if d >= 0:
                            nc.vector.tensor_mul(
                                p_sb[:, qs:qs + 128], p_sb[:, qs:qs + 128], mask_sb)
                        nc.tensor.matmul(
                            outp[:, qs:512],
                            lhsT=(v_sb[kj][:, j * DH:(j + 1) * DH]),
                            rhs=(p_sb[:, qs:512]),
                            start=(kj == 0), stop=(kj == kmax - 1),
                        )
                        nc.tensor.matmul(
                            lp[:, qs:512],
                            lhsT=(onesb_sb[:, 0:1]),
                            rhs=(p_sb[:, qs:512]),
                            start=(kj == 0), stop=(kj == kmax - 1),
                        )
                    l_sb = lrec_pool.tile([1, 512], F32, name="l_sb", tag="l_sb")
                    nc.scalar.activation(l_sb, lp, ACT_COPY)
                    r_sb = lrec_pool.tile([1, 512], F32R, name="r_sb", tag="r_sb")
                    nc.vector.reciprocal(r_sb, l_sb)
                    rp = psR.tile([128, 512], F32, name="rp", tag="rp")
                    nc.tensor.matmul(rp, lhsT=(ones_sb[0:1, :]),
                                     rhs=(r_sb), start=True, stop=True)
                    # DVE may read only one PSUM operand: stage rp in SBUF
                    rbc = lrec_pool.tile([128, 512], F32, name="rbc", tag="rbc")
                    nc.scalar.activation(rbc, rp, ACT_COPY)
                    nc.vector.tensor_mul(outT[j][:, qcols], outp, rbc)

            def outproj_chunk(qc):
                for st4 in range(4):
                    st = qc * 4 + st4
                    for mc in range(NSC):
                        yp_ps = psD.tile([128, 512], F32, name="yp_ps", tag="yp_ps")
                        for j in range(HPC):
                            nc.tensor.matmul(
                                yp_ps,
                                lhsT=(outT[j][:, st * 128:(st + 1) * 128]),
                                rhs=(wo_sb[j][:, mc * 512:(mc + 1) * 512]),
                                start=(j == 0), stop=(j == HPC - 1),
                            )
                        y_sb = y_pool.tile([128, 512], F32, name="y_sb", tag="y_sb")
                        nc.vector.tensor_scalar_mul(y_sb, yp_ps, 1.0)
                        nc.sync.dma_start(
                            out=yp_d[st * 128:(st + 1) * 128, mc * 512:(mc + 1) * 512],
                            in_=y_sb)

            # software pipeline: D(qc) runs one chunk behind C(qc)
            attn_chunk(0)
            attn_chunk(1)
            outproj_chunk(0)
            attn_chunk(2)
            outproj_chunk(1)
            attn_chunk(3)
            outproj_chunk(2)
            outproj_chunk(3)
    return nc


class SpmdRunner:
    def __init__(self, nc, n_cores: int):
        bass2jax.install_neuronx_cc_hook()
        self.nc = nc
        self.n_cores = n_cores
        partition_name = nc.partition_id_tensor.name if nc.partition_id_tensor else None

        in_names, out_names, out_avals = [], [], []
        for alloc in nc.m.functions[0].allocations:
            if not isinstance(alloc, mybir.MemoryLocationSet):
                continue
            name = alloc.memorylocations[0].name
            if alloc.kind == "ExternalInput":
                if name != partition_name:
                    in_names.append(name)
            elif alloc.kind == "ExternalOutput":
                out_names.append(name)
                shape = tuple(alloc.tensor_shape)
                dtype = mybir.dt.np(alloc.dtype)
                out_avals.append(jax.core.ShapedArray(shape, dtype))
        self.in_names = list(in_names)
        self.out_names = out_names
        self.out_avals = out_avals
        n_params = len(in_names)
        all_in_names = in_names + out_names
        if partition_name is not None:
            all_in_names.append(partition_name)

        def _body(*args):
            operands = list(args)
            if partition_name is not None:
                operands.append(partition_id_tensor())
            outs = bass2jax._bass_exec_p.bind(
                *operands,
                out_avals=tuple(out_avals),
                in_names=tuple(all_in_names),
                out_names=tuple(out_names),
                lowering_input_output_aliases=(),
                sim_require_finite=True,
                sim_require_nnan=True,
                nc=nc,
            )
            return tuple(outs)

        devices = jax.devices()[:n_cores]
        self.mesh = Mesh(np.asarray(devices), ("core",))
        in_specs = (PartitionSpec("core"),) * (n_params + len(out_names))
        out_specs = (PartitionSpec("core"),) * len(out_names)
        donate = tuple(range(n_params, n_params + len(out_names)))
        self.jitted = jax.jit(
            shard_map(_body, mesh=self.mesh, in_specs=in_specs,
                      out_specs=out_specs, check_rep=False),
            donate_argnums=donate,
            keep_unused=True,
        )
        self.sharding = jax.sharding.NamedSharding(self.mesh, PartitionSpec("core"))
        # on-device zero allocator for the donated output buffers
        zero_shapes = [(n_cores * av.shape[0], *av.shape[1:]) for av in out_avals]
        zero_dtypes = [av.dtype for av in out_avals]

        def _mk_zeros():
            import jax.numpy as jnp
            return tuple(jnp.zeros(s, d) for s, d in zip(zero_shapes, zero_dtypes))

        self._mk_zeros = jax.jit(_mk_zeros, out_shardings=(self.sharding,) * len(out_avals))
        # Recycled donation buffers: the kernel overwrites every element of
        # each ExternalOutput, so the previous call's outputs can serve as
        # the next call's donated output operands. This keeps run_staged at
        # a single dispatch+block cycle (the per-execute relay round trip
        # dominates wall time; a separate zeros allocation would double it).
        self._outbufs = None

    def concat_inputs(self, in_maps):
        assert len(in_maps) == self.n_cores
        arrs = [
            np.concatenate([np.asarray(in_maps[c][n]) for c in range(self.n_cores)], axis=0)
            for n in self.in_names
        ]
        zeros = [
            np.zeros((self.n_cores * av.shape[0], *av.shape[1:]), av.dtype)
            for av in self.out_avals
        ]
        return arrs, zeros

    def stage(self, in_maps):
        arrs, _ = self.concat_inputs(in_maps)
        staged = [jax.device_put(a, self.sharding) for a in arrs]
        if self._outbufs is None:
            self._outbufs = self._mk_zeros()
        jax.block_until_ready(staged)
        jax.block_until_ready(self._outbufs)
        return staged

    def run_staged(self, staged):
        bufs = self._outbufs
        self._outbufs = None
        if bufs is None:
            bufs = self._mk_zeros()
            jax.block_until_ready(bufs)
        outs = self.jitted(*staged, *bufs)
        jax.block_until_ready(outs)
        self._outbufs = outs
        return outs

    def __call__(self, in_maps):
        staged = self.stage(in_maps)
        outs = self.run_staged(staged)
        res = []
        for c in range(self.n_cores):
            res.append({
                name: np.asarray(outs[i]).reshape(self.n_cores, *self.out_avals[i].shape)[c]
                for i, name in enumerate(self.out_names)
            })
        return res


_NC_CACHE: dict = {}


def _get_runner():
    if "runner" not in _NC_CACHE:
        _NC_CACHE["runner"] = SpmdRunner(_build(), N_CORES)
    return _NC_CACHE["runner"]


def _host_inputs(x, Wq_sem, Wk_sem, Wq_geo, Wk_geo, Wv, Wo):
    bf16 = ml_dtypes.bfloat16
    # RoPE tables: cos replicated on both 32-row geo half-ranges; sin with
    # the rotate-half sign baked in (-sin on rows 64:96, +sin on 96:128).
    inv_freq = 1.0 / (ROPE_BASE ** (np.arange(0, 64, 2, dtype=np.float32) / 64.0))
    t = np.arange(S, dtype=np.float32)
    freqs = np.outer(t, inv_freq)  # [S, 32]
    cosT = np.cos(freqs).T.astype(np.float32)  # [32, S]
    sinT = np.sin(freqs).T.astype(np.float32)
    cs = np.zeros((128, 2 * S), np.float32)
    cs[64:96, :S] = cosT
    cs[96:128, :S] = cosT
    cs[64:96, S:] = -sinT
    cs[96:128, S:] = sinT

    # causal triangle for diagonal blocks: mask[kl, ql] = ql >= kl
    kl = np.arange(128)
    mask = (kl[None, :] >= kl[:, None]).astype(bf16)

    ones = np.ones((128, 128), np.float32)
    onesb = np.ones((128, 128), bf16)

    in_maps = []
    for c in range(N_CORES):
        b, g = divmod(c, 4)
        blocks_q, blocks_k = [], []
        for j in range(HPC):
            h = g * HPC + j
            r64 = slice(h * 64, (h + 1) * 64)
            blocks_q.append(np.concatenate([Wq_sem[r64], Wq_geo[r64]], axis=0) * SCALE)
            blocks_k.append(np.concatenate([Wk_sem[r64], Wk_geo[r64]], axis=0))
        wqk = np.ascontiguousarray(np.concatenate(blocks_q + blocks_k, axis=0).T)
        hv = slice(g * HPC * DH, (g + 1) * HPC * DH)
        wv = np.ascontiguousarray(Wv[hv].T)
        wo = np.ascontiguousarray(Wo[:, hv].T)
        xT = np.ascontiguousarray(x[b].T)
        in_maps.append({
            "xT": xT.astype(bf16),
            "wqk": wqk.astype(bf16),
            "wv": wv.astype(bf16),
            "wo": wo.astype(bf16),
            "cs": cs,
            "mask": mask,
            "ones": ones,
            "onesb": onesb,
        })
    return in_maps


def kernel(x, Wq_sem, Wk_sem, Wq_geo, Wk_geo, Wv, Wo):
    in_maps = _host_inputs(np.asarray(x), np.asarray(Wq_sem), np.asarray(Wk_sem),
                           np.asarray(Wq_geo), np.asarray(Wk_geo),
                           np.asarray(Wv), np.asarray(Wo))
    res = _get_runner()(in_maps)
    y = np.empty((B, S, D), np.float32)
    for b in range(B):
        y[b] = sum(res[b * 4 + g]["yp"] for g in range(4))
    return y


# revision 18
# speedup vs baseline: 1.0074x; 1.0074x over previous
"""DecoupledBottleneckAttention on 8 trn2 NeuronCores.

Sharding: core c -> batch b=c//4, head-group g=c%4 (4 heads/core).
Each core computes q/k/v projections for its heads, causal attention,
and a partial out-projection; the host sums the 4 partials per batch.

v2 layout:
- Single pass over x per 512-column chunk: qk(ob0-3) -> v -> qk(ob4-7)
  matmul passes share the chunk's 16 resident xT tiles, rotating 4+4
  PSUM banks so consecutive passes never wait on bank drains.
- x and all weights are bf16 (fp32 PSUM accumulation); scores and the
  softmax stay fp32; exp probabilities and v are bf16.
- RoPE as 3 DVE ops on [64,512] with the sin sign baked into the table.
- Causal diagonal blocks use variable-width rhs (512/384/256/128) and
  a single [128,128] triangle mask-mul instead of full-width masking.
- Attention (C) and out-projection (D) interleave chunk-wise:
  C0 C1 D0 C2 D1 C3 D2 D3, so D's matmuls hide C's softmax tails.
- exp() skips max-subtraction: logits are bounded (~|6|) by the fixed
  input scale. Denominators come from a ones-column matmul.
"""

import json
from contextlib import ExitStack

import numpy as np
import ml_dtypes

import jax
import concourse.bass as bass
import concourse.mybir as mybir
from concourse.tile import TileContext
from concourse import bass2jax
from concourse.bass2jax import Mesh, PartitionSpec, shard_map, partition_id_tensor

F32 = mybir.dt.float32
F32R = mybir.dt.float32r
BF16 = mybir.dt.bfloat16

B, S, D = 2, 2048, 2048
H = 16
HPC = 4  # heads per core
N_CORES = 8
DH = 128  # per-head q/k/v dim (64 sem + 64 geo; v 128)
ROPE_BASE = 10000.0
SCALE = 1.0 / np.sqrt(128.0)

NSC = S // 512  # 4 s-chunks of 512
NDT = D // 128  # 16 contraction tiles
NST = S // 128  # 16 s-tiles of 128

ACT_COPY = mybir.ActivationFunctionType.Copy
ACT_EXP = mybir.ActivationFunctionType.Exp


def _split_multi_waits(bir: dict) -> dict:
    """walrus here rejects >1 sync waits per instruction; split extras
    into single-wait Drains inserted just before, on the same engine."""
    for fn in bir.get("functions", []):
        for blk in fn.get("blocks", []):
            new_insts = []
            for ins in blk.get("instructions", []):
                si = ins.get("sync_info") or {}
                waits = si.get("on_wait") or []
                if len(waits) > 1:
                    for i, w in enumerate(waits[:-1]):
                        new_insts.append(
                            {
                                "debug": ins.get("debug", 0),
                                "engine": ins["engine"],
                                "ins": [],
                                "name": f"{ins['name']}-w{i}",
                                "opcode": "Drain",
                                "outs": [],
                                "sync_info": {"on_update": [], "on_wait": [w]},
                            }
                        )
                    si["on_wait"] = [waits[-1]]
                new_insts.append(ins)
            blk["instructions"] = new_insts
    return bir


class _PatchedBass(bass.Bass):
    def to_json_bytes(self) -> bytes:
        return json.dumps(_split_multi_waits(json.loads(super().to_json_bytes()))).encode()


def _rd(ap):
    """Bitcast a DRAM-side AP to f32r for DMAs into f32r SBUF tiles."""
    return ap.bitcast(F32R)


def _build():
    nc = _PatchedBass("TRN2", target_bir_lowering=False, debug=False, num_devices=N_CORES)

    xT_d = nc.dram_tensor("xT", [D, S], BF16, kind="ExternalInput")
    wqk_d = nc.dram_tensor("wqk", [D, 8 * 128], BF16, kind="ExternalInput")
    wv_d = nc.dram_tensor("wv", [D, HPC * DH], BF16, kind="ExternalInput")
    wo_d = nc.dram_tensor("wo", [HPC * DH, D], BF16, kind="ExternalInput")
    # rows 64:128 hold cos (cols 0:S) and sign-baked sin (cols S:2S:
    # rows 64:96 = -sin, rows 96:128 = +sin); rows 0:64 unused.
    cs_d = nc.dram_tensor("cs", [128, 2 * S], F32, kind="ExternalInput")
    mask_d = nc.dram_tensor("mask", [128, 128], BF16, kind="ExternalInput")
    ones_d = nc.dram_tensor("ones", [128, 128], F32, kind="ExternalInput")
    onesb_d = nc.dram_tensor("onesb", [128, 128], BF16, kind="ExternalInput")
    yp_d = nc.dram_tensor("yp", [S, D], F32, kind="ExternalOutput")

    with TileContext(nc) as tc, ExitStack() as ctx, \
         nc.allow_low_precision(reason="float32r tiles are 4-byte fp32 at rest"):
        pers = ctx.enter_context(tc.tile_pool(name="pers", bufs=1))
        # qkT[0..3] = per-head qT [128 dims, S]; qkT[4..7] = kT
        qkT = [pers.tile([128, S], F32R, name=f"qkT{i}", tag=f"qkT{i}") for i in range(8)]
        v_sb = [pers.tile([128, HPC * DH], BF16, name=f"v{st}", tag=f"v{st}")
                for st in range(NST)]
        cs_sb = pers.tile([128, 2 * S], F32, name="cs_sb", tag="cs_sb")
        ones_sb = pers.tile([128, 128], F32R, name="ones_sb", tag="ones_sb")
        onesb_sb = pers.tile([128, 128], BF16, name="onesb_sb", tag="onesb_sb")
        mask_sb = pers.tile([128, 128], BF16, name="mask_sb", tag="mask_sb")

        # ------------- Phase A+B: q/k/v projections, one x pass --------
        with tc.tile_pool(name="wqk", bufs=1) as wqk_pool, \
             tc.tile_pool(name="wv", bufs=1) as wv_pool, \
             tc.tile_pool(name="xt", bufs=2) as xt_pool, \
             tc.tile_pool(name="rope", bufs=4) as rope_pool, \
             tc.tile_pool(name="psQK", bufs=1, space="PSUM") as psQK, \
             tc.tile_pool(name="psV", bufs=1, space="PSUM") as psV:
            wqk_sb = [wqk_pool.tile([128, 8 * 128], BF16, name=f"wqk{dt}", tag=f"wqk{dt}")
                      for dt in range(NDT)]
            wv_sb = [wv_pool.tile([128, HPC * DH], BF16, name=f"wv{dt}", tag=f"wv{dt}")
                     for dt in range(NDT)]

            def xt_tiles(sc):
                tiles = [xt_pool.tile([128, 512], BF16, name="xt_t", tag=f"xt{dt}")
                         for dt in range(NDT)]
                for dt in range(NDT):
                    nc.sync.dma_start(
                        out=tiles[dt],
                        in_=xT_d[dt * 128:(dt + 1) * 128, sc * 512:(sc + 1) * 512])
                return tiles

            # chunk-0 stream: weights interleaved with x tiles so the first
            # matmul's operands arrive first; bulk tables after.
            xt_cur = [xt_pool.tile([128, 512], BF16, name="xt_t", tag=f"xt{dt}")
                      for dt in range(NDT)]
            for dt in range(NDT):
                nc.sync.dma_start(out=wqk_sb[dt], in_=wqk_d[dt * 128:(dt + 1) * 128, :])
                nc.sync.dma_start(
                    out=xt_cur[dt], in_=xT_d[dt * 128:(dt + 1) * 128, 0:512])
            for dt in range(NDT):
                nc.sync.dma_start(out=wv_sb[dt], in_=wv_d[dt * 128:(dt + 1) * 128, :])
            nc.sync.dma_start(out=cs_sb, in_=cs_d[:, :])
            nc.sync.dma_start(out=ones_sb, in_=_rd(ones_d[:, :]))
            nc.sync.dma_start(out=onesb_sb, in_=onesb_d[:, :])
            nc.sync.dma_start(out=mask_sb, in_=mask_d[:, :])

            Am, Bm = slice(64, 96), slice(96, 128)

            def rope_out(ob, cols):
                """PSUM qk tile -> qkT[ob] with RoPE on the geo rows."""
                ps = ps_qk[ob % 4]
                dst = qkT[ob]
                nc.scalar.activation(dst[0:64, cols], ps[0:64, :], ACT_COPY)
                stage = rope_pool.tile([128, 512], F32, name="ropest", tag="ropest")
                sw = rope_pool.tile([128, 512], F32, name="ropesw", tag="ropesw")
                # stage copy on DVE: halves the ACT backlog that would queue
                # the attention phase's first exp() behind rope drains.
                nc.vector.tensor_scalar_mul(stage[64:128, :], ps[64:128, :], 1.0)
                nc.sync.dma_start(out=sw[Am, :], in_=stage[Bm, :])  # x2 -> A rows
                nc.sync.dma_start(out=sw[Bm, :], in_=stage[Am, :])  # x1 -> B rows
                csc = cs_sb[64:128, sc * 512:(sc + 1) * 512]
                sns = cs_sb[64:128, S + sc * 512:S + (sc + 1) * 512]
                g = slice(64, 128)
                tp = rope_pool.tile([128, 512], F32, name="ropetp", tag="ropetp")
                nc.vector.tensor_mul(tp[g, :], stage[g, :], csc)     # [x1;x2]*cos
                nc.vector.tensor_mul(sw[g, :], sw[g, :], sns)        # [x2;x1]*(-/+sin)
                nc.vector.tensor_add(dst[g, cols], tp[g, :], sw[g, :])

            for sc in range(NSC):
                cols = slice(sc * 512, (sc + 1) * 512)
                if sc + 1 < NSC:
                    xt_next = xt_tiles(sc + 1)
                else:
                    xt_next = None
                # pass 1: q/k heads 0-3
                ps_qk = [psQK.tile([128, 512], F32, name=f"psqk{i}", tag=f"psqk{i}")
                         for i in range(4)]
                for dt in range(NDT):
                    for ob in range(4):
                        nc.tensor.matmul(
                            ps_qk[ob],
                            lhsT=(wqk_sb[dt][:, ob * 128:(ob + 1) * 128]),
                            rhs=(xt_cur[dt]),
                            start=(dt == 0), stop=(dt == NDT - 1),
                        )
                for ob in range(4):
                    rope_out(ob, cols)
                # pass 2: v projection (natural [s, head-dim] layout)
                psv = [psV.tile([128, HPC * DH], F32, name=f"psv{st}", tag=f"psv{st}")
                       for st in range(4)]
                for dt in range(NDT):
                    for st in range(4):
                        nc.tensor.matmul(
                            psv[st],
                            lhsT=(xt_cur[dt][:, st * 128:(st + 1) * 128]),
                            rhs=(wv_sb[dt]),
                            start=(dt == 0), stop=(dt == NDT - 1),
                        )
                for st in range(4):
                    nc.scalar.activation(v_sb[sc * 4 + st], psv[st], ACT_COPY)
                # pass 3: q/k heads 4-7 (reuses pass-1 banks, drained by now)
                ps_qk = [psQK.tile([128, 512], F32, name=f"psqk{i}", tag=f"psqk{i}")
                         for i in range(4)]
                for dt in range(NDT):
                    for ob in range(4):
                        nc.tensor.matmul(
                            ps_qk[ob],
                            lhsT=(wqk_sb[dt][:, (ob + 4) * 128:(ob + 5) * 128]),
                            rhs=(xt_cur[dt]),
                            start=(dt == 0), stop=(dt == NDT - 1),
                        )
                for ob in range(4):
                    rope_out(ob + 4, cols)
                if xt_next is not None:
                    xt_cur = xt_next

        # ------------- Phase C+D: attention + out-projection -----------
        with tc.tile_pool(name="wo", bufs=1) as wo_pool, \
             tc.tile_pool(name="outT", bufs=1) as outT_pool, \
             tc.tile_pool(name="attn", bufs=5) as attn_pool, \
             tc.tile_pool(name="lrec", bufs=2) as lrec_pool, \
             tc.tile_pool(name="ysb", bufs=5) as y_pool, \
             tc.tile_pool(name="psL", bufs=1, space="PSUM") as psL, \
             tc.tile_pool(name="psR", bufs=1, space="PSUM") as psR, \
             tc.tile_pool(name="psD", bufs=2, space="PSUM") as psD, \
             tc.tile_pool(name="psST", bufs=2, space="PSUM") as psST, \
             tc.tile_pool(name="psOut", bufs=2, space="PSUM") as psOut:
            wo_sb = [wo_pool.tile([128, D], BF16, name=f"wo{j}", tag=f"wo{j}")
                     for j in range(HPC)]
            for j in range(HPC):
                nc.sync.dma_start(out=wo_sb[j], in_=wo_d[j * 128:(j + 1) * 128, :])
            outT = [outT_pool.tile([128, S], BF16, name=f"outT{j}", tag=f"outT{j}")
                    for j in range(HPC)]

            def attn_chunk(qc):
                qcols = slice(qc * 512, (qc + 1) * 512)
                kmax = qc * 4 + 4
                for j in range(HPC):
                    outp = psOut.tile([128, 512], F32, name="outp", tag="outp")
                    lp = psL.tile([1, 512], F32, name="lp", tag="lp")
                    for kj in range(kmax):
                        d = kj - qc * 4
                        qs = 0 if d < 0 else d * 128
                        w = 512 - qs
                        st_ps = psST.tile([128, 512], F32, name="st_ps", tag="st_ps")
                        nc.tensor.matmul(
                            st_ps[:, 0:w],
                            lhsT=(qkT[4 + j][:, kj * 128:(kj + 1) * 128]),
                            rhs=(qkT[j][:, qc * 512 + qs:(qc + 1) * 512]),
                            start=True, stop=True,
                        )
                        p_sb = attn_pool.tile([128, 512], BF16, name="p_sb", tag="p_sb")
                        nc.scalar.activation(p_sb[:, qs:512], st_ps[:, 0:w], ACT_EXP)
                        if d >= 0:
                            nc.vector.tensor_mul(
                                p_sb[:, qs:qs + 128], p_sb[:, qs:qs + 128], mask_sb)
                        nc.tensor.matmul(
                            outp[:, qs:512],
                            lhsT=(v_sb[kj][:, j * DH:(j + 1) * DH]),
                            rhs=(p_sb[:, qs:512]),
                            start=(kj == 0), stop=(kj == kmax - 1),
                        )
                        nc.tensor.matmul(
                            lp[:, qs:512],
                            lhsT=(onesb_sb[:, 0:1]),
                            rhs=(p_sb[:, qs:512]),
                            start=(kj == 0), stop=(kj == kmax - 1),
                        )
                    l_sb = lrec_pool.tile([1, 512], F32, name="l_sb", tag="l_sb")
                    nc.scalar.activation(l_sb, lp, ACT_COPY)
                    r_sb = lrec_pool.tile([1, 512], F32R, name="r_sb", tag="r_sb")
                    nc.vector.reciprocal(r_sb, l_sb)
                    rp = psR.tile([128, 512], F32, name="rp", tag="rp")
                    nc.tensor.matmul(rp, lhsT=(ones_sb[0:1, :]),
                                     rhs=(r_sb), start=True, stop=True)
                    # DVE may read only one PSUM operand: stage rp in SBUF
                    rbc = lrec_pool.tile([128, 512], F32, name="rbc", tag="rbc")
                    nc.scalar.activation(rbc, rp, ACT_COPY)
                    nc.vector.tensor_mul(outT[j][:, qcols], outp, rbc)

            def outproj_chunk(qc):
                for st4 in range(4):
                    st = qc * 4 + st4
                    for mc in range(NSC):
                        yp_ps = psD.tile([128, 512], F32, name="yp_ps", tag="yp_ps")
                        for j in range(HPC):
                            nc.tensor.matmul(
                                yp_ps,
                                lhsT=(outT[j][:, st * 128:(st + 1) * 128]),
                                rhs=(wo_sb[j][:, mc * 512:(mc + 1) * 512]),
                                start=(j == 0), stop=(j == HPC - 1),
                            )
                        y_sb = y_pool.tile([128, 512], F32, name="y_sb", tag="y_sb")
                        nc.vector.tensor_scalar_mul(y_sb, yp_ps, 1.0)
                        nc.sync.dma_start(
                            out=yp_d[st * 128:(st + 1) * 128, mc * 512:(mc + 1) * 512],
                            in_=y_sb)

            # software pipeline: D(qc) runs one chunk behind C(qc);
            # chunk-0 attention gets scheduler priority so its exp() ops
            # are not queued behind the projection phase's last rope drains
            with tc.high_priority():
                attn_chunk(0)
            attn_chunk(1)
            outproj_chunk(0)
            attn_chunk(2)
            outproj_chunk(1)
            attn_chunk(3)
            outproj_chunk(2)
            outproj_chunk(3)
    return nc


class SpmdRunner:
    def __init__(self, nc, n_cores: int):
        bass2jax.install_neuronx_cc_hook()
        self.nc = nc
        self.n_cores = n_cores
        partition_name = nc.partition_id_tensor.name if nc.partition_id_tensor else None

        in_names, out_names, out_avals = [], [], []
        for alloc in nc.m.functions[0].allocations:
            if not isinstance(alloc, mybir.MemoryLocationSet):
                continue
            name = alloc.memorylocations[0].name
            if alloc.kind == "ExternalInput":
                if name != partition_name:
                    in_names.append(name)
            elif alloc.kind == "ExternalOutput":
                out_names.append(name)
                shape = tuple(alloc.tensor_shape)
                dtype = mybir.dt.np(alloc.dtype)
                out_avals.append(jax.core.ShapedArray(shape, dtype))
        self.in_names = list(in_names)
        self.out_names = out_names
        self.out_avals = out_avals
        n_params = len(in_names)
        all_in_names = in_names + out_names
        if partition_name is not None:
            all_in_names.append(partition_name)

        def _body(*args):
            operands = list(args)
            if partition_name is not None:
                operands.append(partition_id_tensor())
            outs = bass2jax._bass_exec_p.bind(
                *operands,
                out_avals=tuple(out_avals),
                in_names=tuple(all_in_names),
                out_names=tuple(out_names),
                lowering_input_output_aliases=(),
                sim_require_finite=True,
                sim_require_nnan=True,
                nc=nc,
            )
            return tuple(outs)

        devices = jax.devices()[:n_cores]
        self.mesh = Mesh(np.asarray(devices), ("core",))
        in_specs = (PartitionSpec("core"),) * (n_params + len(out_names))
        out_specs = (PartitionSpec("core"),) * len(out_names)
        donate = tuple(range(n_params, n_params + len(out_names)))
        self.jitted = jax.jit(
            shard_map(_body, mesh=self.mesh, in_specs=in_specs,
                      out_specs=out_specs, check_rep=False),
            donate_argnums=donate,
            keep_unused=True,
        )
        self.sharding = jax.sharding.NamedSharding(self.mesh, PartitionSpec("core"))
        # on-device zero allocator for the donated output buffers
        zero_shapes = [(n_cores * av.shape[0], *av.shape[1:]) for av in out_avals]
        zero_dtypes = [av.dtype for av in out_avals]

        def _mk_zeros():
            import jax.numpy as jnp
            return tuple(jnp.zeros(s, d) for s, d in zip(zero_shapes, zero_dtypes))

        self._mk_zeros = jax.jit(_mk_zeros, out_shardings=(self.sharding,) * len(out_avals))
        # Recycled donation buffers: the kernel overwrites every element of
        # each ExternalOutput, so the previous call's outputs can serve as
        # the next call's donated output operands. This keeps run_staged at
        # a single dispatch+block cycle (the per-execute relay round trip
        # dominates wall time; a separate zeros allocation would double it).
        self._outbufs = None

    def concat_inputs(self, in_maps):
        assert len(in_maps) == self.n_cores
        arrs = [
            np.concatenate([np.asarray(in_maps[c][n]) for c in range(self.n_cores)], axis=0)
            for n in self.in_names
        ]
        zeros = [
            np.zeros((self.n_cores * av.shape[0], *av.shape[1:]), av.dtype)
            for av in self.out_avals
        ]
        return arrs, zeros

    def stage(self, in_maps):
        arrs, _ = self.concat_inputs(in_maps)
        staged = [jax.device_put(a, self.sharding) for a in arrs]
        if self._outbufs is None:
            self._outbufs = self._mk_zeros()
        jax.block_until_ready(staged)
        jax.block_until_ready(self._outbufs)
        return staged

    def run_staged(self, staged):
        bufs = self._outbufs
        self._outbufs = None
        if bufs is None:
            bufs = self._mk_zeros()
            jax.block_until_ready(bufs)
        outs = self.jitted(*staged, *bufs)
        jax.block_until_ready(outs)
        self._outbufs = outs
        return outs

    def __call__(self, in_maps):
        staged = self.stage(in_maps)
        outs = self.run_staged(staged)
        res = []
        for c in range(self.n_cores):
            res.append({
                name: np.asarray(outs[i]).reshape(self.n_cores, *self.out_avals[i].shape)[c]
                for i, name in enumerate(self.out_names)
            })
        return res


_NC_CACHE: dict = {}


def _get_runner():
    if "runner" not in _NC_CACHE:
        _NC_CACHE["runner"] = SpmdRunner(_build(), N_CORES)
    return _NC_CACHE["runner"]


def _host_inputs(x, Wq_sem, Wk_sem, Wq_geo, Wk_geo, Wv, Wo):
    bf16 = ml_dtypes.bfloat16
    # RoPE tables: cos replicated on both 32-row geo half-ranges; sin with
    # the rotate-half sign baked in (-sin on rows 64:96, +sin on 96:128).
    inv_freq = 1.0 / (ROPE_BASE ** (np.arange(0, 64, 2, dtype=np.float32) / 64.0))
    t = np.arange(S, dtype=np.float32)
    freqs = np.outer(t, inv_freq)  # [S, 32]
    cosT = np.cos(freqs).T.astype(np.float32)  # [32, S]
    sinT = np.sin(freqs).T.astype(np.float32)
    cs = np.zeros((128, 2 * S), np.float32)
    cs[64:96, :S] = cosT
    cs[96:128, :S] = cosT
    cs[64:96, S:] = -sinT
    cs[96:128, S:] = sinT

    # causal triangle for diagonal blocks: mask[kl, ql] = ql >= kl
    kl = np.arange(128)
    mask = (kl[None, :] >= kl[:, None]).astype(bf16)

    ones = np.ones((128, 128), np.float32)
    onesb = np.ones((128, 128), bf16)

    in_maps = []
    for c in range(N_CORES):
        b, g = divmod(c, 4)
        blocks_q, blocks_k = [], []
        for j in range(HPC):
            h = g * HPC + j
            r64 = slice(h * 64, (h + 1) * 64)
            blocks_q.append(np.concatenate([Wq_sem[r64], Wq_geo[r64]], axis=0) * SCALE)
            blocks_k.append(np.concatenate([Wk_sem[r64], Wk_geo[r64]], axis=0))
        wqk = np.ascontiguousarray(np.concatenate(blocks_q + blocks_k, axis=0).T)
        hv = slice(g * HPC * DH, (g + 1) * HPC * DH)
        wv = np.ascontiguousarray(Wv[hv].T)
        wo = np.ascontiguousarray(Wo[:, hv].T)
        xT = np.ascontiguousarray(x[b].T)
        in_maps.append({
            "xT": xT.astype(bf16),
            "wqk": wqk.astype(bf16),
            "wv": wv.astype(bf16),
            "wo": wo.astype(bf16),
            "cs": cs,
            "mask": mask,
            "ones": ones,
            "onesb": onesb,
        })
    return in_maps


def kernel(x, Wq_sem, Wk_sem, Wq_geo, Wk_geo, Wv, Wo):
    in_maps = _host_inputs(np.asarray(x), np.asarray(Wq_sem), np.asarray(Wk_sem),
                           np.asarray(Wq_geo), np.asarray(Wk_geo),
                           np.asarray(Wv), np.asarray(Wo))
    res = _get_runner()(in_maps)
    y = np.empty((B, S, D), np.float32)
    for b in range(B):
        y[b] = sum(res[b * 4 + g]["yp"] for g in range(4))
    return y


# revision 19
# speedup vs baseline: 1.0094x; 1.0020x over previous
"""DecoupledBottleneckAttention on 8 trn2 NeuronCores.

Sharding: core c -> batch b=c//4, head-group g=c%4 (4 heads/core).
Each core computes q/k/v projections for its heads, causal attention,
and a partial out-projection; the host sums the 4 partials per batch.

v2 layout:
- Single pass over x per 512-column chunk: qk(ob0-3) -> v -> qk(ob4-7)
  matmul passes share the chunk's 16 resident xT tiles, rotating 4+4
  PSUM banks so consecutive passes never wait on bank drains.
- x and all weights are bf16 (fp32 PSUM accumulation); scores and the
  softmax stay fp32; exp probabilities and v are bf16.
- RoPE as 3 DVE ops on [64,512] with the sin sign baked into the table.
- Causal diagonal blocks use variable-width rhs (512/384/256/128) and
  a single [128,128] triangle mask-mul instead of full-width masking.
- Attention (C) and out-projection (D) interleave chunk-wise:
  C0 C1 D0 C2 D1 C3 D2 D3, so D's matmuls hide C's softmax tails.
- exp() skips max-subtraction: logits are bounded (~|6|) by the fixed
  input scale. Denominators come from a ones-column matmul.
"""

import json
from contextlib import ExitStack

import numpy as np
import ml_dtypes

import jax
import concourse.bass as bass
import concourse.mybir as mybir
from concourse.tile import TileContext
from concourse import bass2jax
from concourse.bass2jax import Mesh, PartitionSpec, shard_map, partition_id_tensor

F32 = mybir.dt.float32
F32R = mybir.dt.float32r
BF16 = mybir.dt.bfloat16

B, S, D = 2, 2048, 2048
H = 16
HPC = 4  # heads per core
N_CORES = 8
DH = 128  # per-head q/k/v dim (64 sem + 64 geo; v 128)
ROPE_BASE = 10000.0
SCALE = 1.0 / np.sqrt(128.0)

NSC = S // 512  # 4 s-chunks of 512
NDT = D // 128  # 16 contraction tiles
NST = S // 128  # 16 s-tiles of 128

ACT_COPY = mybir.ActivationFunctionType.Copy
ACT_EXP = mybir.ActivationFunctionType.Exp


def _split_multi_waits(bir: dict) -> dict:
    """walrus here rejects >1 sync waits per instruction; split extras
    into single-wait Drains inserted just before, on the same engine."""
    for fn in bir.get("functions", []):
        for blk in fn.get("blocks", []):
            new_insts = []
            for ins in blk.get("instructions", []):
                si = ins.get("sync_info") or {}
                waits = si.get("on_wait") or []
                if len(waits) > 1:
                    for i, w in enumerate(waits[:-1]):
                        new_insts.append(
                            {
                                "debug": ins.get("debug", 0),
                                "engine": ins["engine"],
                                "ins": [],
                                "name": f"{ins['name']}-w{i}",
                                "opcode": "Drain",
                                "outs": [],
                                "sync_info": {"on_update": [], "on_wait": [w]},
                            }
                        )
                    si["on_wait"] = [waits[-1]]
                new_insts.append(ins)
            blk["instructions"] = new_insts
    return bir


class _PatchedBass(bass.Bass):
    def to_json_bytes(self) -> bytes:
        return json.dumps(_split_multi_waits(json.loads(super().to_json_bytes()))).encode()


def _rd(ap):
    """Bitcast a DRAM-side AP to f32r for DMAs into f32r SBUF tiles."""
    return ap.bitcast(F32R)


def _build():
    nc = _PatchedBass("TRN2", target_bir_lowering=False, debug=False, num_devices=N_CORES)

    xT_d = nc.dram_tensor("xT", [D, S], BF16, kind="ExternalInput")
    wqk_d = nc.dram_tensor("wqk", [D, 8 * 128], BF16, kind="ExternalInput")
    wv_d = nc.dram_tensor("wv", [D, HPC * DH], BF16, kind="ExternalInput")
    wo_d = nc.dram_tensor("wo", [HPC * DH, D], BF16, kind="ExternalInput")
    # rows 64:128 hold cos (cols 0:S) and sign-baked sin (cols S:2S:
    # rows 64:96 = -sin, rows 96:128 = +sin); rows 0:64 unused.
    cs_d = nc.dram_tensor("cs", [128, 2 * S], F32, kind="ExternalInput")
    mask_d = nc.dram_tensor("mask", [128, 128], BF16, kind="ExternalInput")
    ones_d = nc.dram_tensor("ones", [128, 128], F32, kind="ExternalInput")
    onesb_d = nc.dram_tensor("onesb", [128, 128], BF16, kind="ExternalInput")
    yp_d = nc.dram_tensor("yp", [S, D], F32, kind="ExternalOutput")

    with TileContext(nc) as tc, ExitStack() as ctx, \
         nc.allow_low_precision(reason="float32r tiles are 4-byte fp32 at rest"):
        pers = ctx.enter_context(tc.tile_pool(name="pers", bufs=1))
        # qkT[0..3] = per-head qT [128 dims, S]; qkT[4..7] = kT
        qkT = [pers.tile([128, S], F32R, name=f"qkT{i}", tag=f"qkT{i}") for i in range(8)]
        v_sb = [pers.tile([128, HPC * DH], BF16, name=f"v{st}", tag=f"v{st}")
                for st in range(NST)]
        cs_sb = pers.tile([128, 2 * S], F32, name="cs_sb", tag="cs_sb")
        ones_sb = pers.tile([128, 128], F32R, name="ones_sb", tag="ones_sb")
        onesb_sb = pers.tile([128, 128], BF16, name="onesb_sb", tag="onesb_sb")
        mask_sb = pers.tile([128, 128], BF16, name="mask_sb", tag="mask_sb")

        # ------------- Phase A+B: q/k/v projections, one x pass --------
        with tc.tile_pool(name="wqk", bufs=1) as wqk_pool, \
             tc.tile_pool(name="wv", bufs=1) as wv_pool, \
             tc.tile_pool(name="xt", bufs=2) as xt_pool, \
             tc.tile_pool(name="rope", bufs=4) as rope_pool, \
             tc.tile_pool(name="psQK", bufs=1, space="PSUM") as psQK, \
             tc.tile_pool(name="psV", bufs=1, space="PSUM") as psV:
            wqk_sb = [wqk_pool.tile([128, 8 * 128], BF16, name=f"wqk{dt}", tag=f"wqk{dt}")
                      for dt in range(NDT)]
            wv_sb = [wv_pool.tile([128, HPC * DH], BF16, name=f"wv{dt}", tag=f"wv{dt}")
                     for dt in range(NDT)]

            def xt_tiles(sc):
                tiles = [xt_pool.tile([128, 512], BF16, name="xt_t", tag=f"xt{dt}")
                         for dt in range(NDT)]
                for dt in range(NDT):
                    nc.sync.dma_start(
                        out=tiles[dt],
                        in_=xT_d[dt * 128:(dt + 1) * 128, sc * 512:(sc + 1) * 512])
                return tiles

            # chunk-0 stream: weights interleaved with x tiles so the first
            # matmul's operands arrive first; bulk tables after.
            xt_cur = [xt_pool.tile([128, 512], BF16, name="xt_t", tag=f"xt{dt}")
                      for dt in range(NDT)]
            for dt in range(NDT):
                nc.sync.dma_start(out=wqk_sb[dt], in_=wqk_d[dt * 128:(dt + 1) * 128, :])
                nc.sync.dma_start(
                    out=xt_cur[dt], in_=xT_d[dt * 128:(dt + 1) * 128, 0:512])
            for dt in range(NDT):
                nc.sync.dma_start(out=wv_sb[dt], in_=wv_d[dt * 128:(dt + 1) * 128, :])
            nc.sync.dma_start(out=cs_sb, in_=cs_d[:, :])
            nc.sync.dma_start(out=ones_sb, in_=_rd(ones_d[:, :]))
            nc.sync.dma_start(out=onesb_sb, in_=onesb_d[:, :])
            nc.sync.dma_start(out=mask_sb, in_=mask_d[:, :])

            Am, Bm = slice(64, 96), slice(96, 128)

            def rope_out(ob, cols):
                """PSUM qk tile -> qkT[ob] with RoPE on the geo rows."""
                ps = ps_qk[ob % 4]
                dst = qkT[ob]
                nc.scalar.activation(dst[0:64, cols], ps[0:64, :], ACT_COPY)
                stage = rope_pool.tile([128, 512], F32, name="ropest", tag="ropest")
                sw = rope_pool.tile([128, 512], F32, name="ropesw", tag="ropesw")
                # stage copy on DVE: halves the ACT backlog that would queue
                # the attention phase's first exp() behind rope drains.
                nc.vector.tensor_scalar_mul(stage[64:128, :], ps[64:128, :], 1.0)
                nc.sync.dma_start(out=sw[Am, :], in_=stage[Bm, :])  # x2 -> A rows
                nc.sync.dma_start(out=sw[Bm, :], in_=stage[Am, :])  # x1 -> B rows
                csc = cs_sb[64:128, sc * 512:(sc + 1) * 512]
                sns = cs_sb[64:128, S + sc * 512:S + (sc + 1) * 512]
                g = slice(64, 128)
                tp = rope_pool.tile([128, 512], F32, name="ropetp", tag="ropetp")
                nc.vector.tensor_mul(tp[g, :], stage[g, :], csc)     # [x1;x2]*cos
                nc.vector.tensor_mul(sw[g, :], sw[g, :], sns)        # [x2;x1]*(-/+sin)
                nc.vector.tensor_add(dst[g, cols], tp[g, :], sw[g, :])

            for sc in range(NSC):
                cols = slice(sc * 512, (sc + 1) * 512)
                if sc + 1 < NSC:
                    xt_next = xt_tiles(sc + 1)
                else:
                    xt_next = None
                # pass 1: q/k heads 0-3
                ps_qk = [psQK.tile([128, 512], F32, name=f"psqk{i}", tag=f"psqk{i}")
                         for i in range(4)]
                for dt in range(NDT):
                    for ob in range(4):
                        nc.tensor.matmul(
                            ps_qk[ob],
                            lhsT=(wqk_sb[dt][:, ob * 128:(ob + 1) * 128]),
                            rhs=(xt_cur[dt]),
                            start=(dt == 0), stop=(dt == NDT - 1),
                        )
                for ob in range(4):
                    rope_out(ob, cols)
                # pass 2: v projection (natural [s, head-dim] layout)
                psv = [psV.tile([128, HPC * DH], F32, name=f"psv{st}", tag=f"psv{st}")
                       for st in range(4)]
                for dt in range(NDT):
                    for st in range(4):
                        nc.tensor.matmul(
                            psv[st],
                            lhsT=(xt_cur[dt][:, st * 128:(st + 1) * 128]),
                            rhs=(wv_sb[dt]),
                            start=(dt == 0), stop=(dt == NDT - 1),
                        )
                for st in range(4):
                    nc.scalar.activation(v_sb[sc * 4 + st], psv[st], ACT_COPY)
                # pass 3: q/k heads 4-7 (reuses pass-1 banks, drained by now)
                ps_qk = [psQK.tile([128, 512], F32, name=f"psqk{i}", tag=f"psqk{i}")
                         for i in range(4)]
                for dt in range(NDT):
                    for ob in range(4):
                        nc.tensor.matmul(
                            ps_qk[ob],
                            lhsT=(wqk_sb[dt][:, (ob + 4) * 128:(ob + 5) * 128]),
                            rhs=(xt_cur[dt]),
                            start=(dt == 0), stop=(dt == NDT - 1),
                        )
                for ob in range(4):
                    rope_out(ob + 4, cols)
                if xt_next is not None:
                    xt_cur = xt_next

        # ------------- Phase C+D: attention + out-projection -----------
        with tc.tile_pool(name="wo", bufs=1) as wo_pool, \
             tc.tile_pool(name="outT", bufs=1) as outT_pool, \
             tc.tile_pool(name="attn", bufs=5) as attn_pool, \
             tc.tile_pool(name="lrec", bufs=2) as lrec_pool, \
             tc.tile_pool(name="ysb", bufs=5) as y_pool, \
             tc.tile_pool(name="psL", bufs=1, space="PSUM") as psL, \
             tc.tile_pool(name="psR", bufs=1, space="PSUM") as psR, \
             tc.tile_pool(name="psD", bufs=2, space="PSUM") as psD, \
             tc.tile_pool(name="psST", bufs=2, space="PSUM") as psST, \
             tc.tile_pool(name="psOut", bufs=2, space="PSUM") as psOut:
            wo_sb = [wo_pool.tile([128, D], BF16, name=f"wo{j}", tag=f"wo{j}")
                     for j in range(HPC)]
            for j in range(HPC):
                nc.sync.dma_start(out=wo_sb[j], in_=wo_d[j * 128:(j + 1) * 128, :])
            outT = [outT_pool.tile([128, S], BF16, name=f"outT{j}", tag=f"outT{j}")
                    for j in range(HPC)]

            def attn_chunk(qc):
                qcols = slice(qc * 512, (qc + 1) * 512)
                kmax = qc * 4 + 4
                for j in range(HPC):
                    outp = psOut.tile([128, 512], F32, name="outp", tag="outp")
                    lp = psL.tile([1, 512], F32, name="lp", tag="lp")
                    for kj in range(kmax):
                        d = kj - qc * 4
                        qs = 0 if d < 0 else d * 128
                        w = 512 - qs
                        st_ps = psST.tile([128, 512], F32, name="st_ps", tag="st_ps")
                        nc.tensor.matmul(
                            st_ps[:, 0:w],
                            lhsT=(qkT[4 + j][:, kj * 128:(kj + 1) * 128]),
                            rhs=(qkT[j][:, qc * 512 + qs:(qc + 1) * 512]),
                            start=True, stop=True,
                        )
                        p_sb = attn_pool.tile([128, 512], BF16, name="p_sb", tag="p_sb")
                        nc.scalar.activation(p_sb[:, qs:512], st_ps[:, 0:w], ACT_EXP)
                        if d >= 0:
                            nc.vector.tensor_mul(
                                p_sb[:, qs:qs + 128], p_sb[:, qs:qs + 128], mask_sb)
                        nc.tensor.matmul(
                            outp[:, qs:512],
                            lhsT=(v_sb[kj][:, j * DH:(j + 1) * DH]),
                            rhs=(p_sb[:, qs:512]),
                            start=(kj == 0), stop=(kj == kmax - 1),
                        )
                        nc.tensor.matmul(
                            lp[:, qs:512],
                            lhsT=(onesb_sb[:, 0:1]),
                            rhs=(p_sb[:, qs:512]),
                            start=(kj == 0), stop=(kj == kmax - 1),
                        )
                    # DVE reads the denominator PSUM directly (its one
                    # allowed PSUM operand) - no ACT staging copy needed
                    r_sb = lrec_pool.tile([1, 512], F32R, name="r_sb", tag="r_sb")
                    nc.vector.reciprocal(r_sb, lp)
                    rp = psR.tile([128, 512], F32, name="rp", tag="rp")
                    nc.tensor.matmul(rp, lhsT=(ones_sb[0:1, :]),
                                     rhs=(r_sb), start=True, stop=True)
                    # DVE may read only one PSUM operand: stage rp in SBUF
                    rbc = lrec_pool.tile([128, 512], F32, name="rbc", tag="rbc")
                    nc.scalar.activation(rbc, rp, ACT_COPY)
                    nc.vector.tensor_mul(outT[j][:, qcols], outp, rbc)

            def outproj_chunk(qc):
                for st4 in range(4):
                    st = qc * 4 + st4
                    for mc in range(NSC):
                        yp_ps = psD.tile([128, 512], F32, name="yp_ps", tag="yp_ps")
                        for j in range(HPC):
                            nc.tensor.matmul(
                                yp_ps,
                                lhsT=(outT[j][:, st * 128:(st + 1) * 128]),
                                rhs=(wo_sb[j][:, mc * 512:(mc + 1) * 512]),
                                start=(j == 0), stop=(j == HPC - 1),
                            )
                        y_sb = y_pool.tile([128, 512], F32, name="y_sb", tag="y_sb")
                        nc.vector.tensor_scalar_mul(y_sb, yp_ps, 1.0)
                        nc.sync.dma_start(
                            out=yp_d[st * 128:(st + 1) * 128, mc * 512:(mc + 1) * 512],
                            in_=y_sb)

            # software pipeline: D(qc) runs one chunk behind C(qc);
            # chunk-0 attention gets scheduler priority so its exp() ops
            # are not queued behind the projection phase's last rope drains
            with tc.high_priority():
                attn_chunk(0)
            attn_chunk(1)
            outproj_chunk(0)
            attn_chunk(2)
            outproj_chunk(1)
            attn_chunk(3)
            outproj_chunk(2)
            outproj_chunk(3)
    return nc


class SpmdRunner:
    def __init__(self, nc, n_cores: int):
        bass2jax.install_neuronx_cc_hook()
        self.nc = nc
        self.n_cores = n_cores
        partition_name = nc.partition_id_tensor.name if nc.partition_id_tensor else None

        in_names, out_names, out_avals = [], [], []
        for alloc in nc.m.functions[0].allocations:
            if not isinstance(alloc, mybir.MemoryLocationSet):
                continue
            name = alloc.memorylocations[0].name
            if alloc.kind == "ExternalInput":
                if name != partition_name:
                    in_names.append(name)
            elif alloc.kind == "ExternalOutput":
                out_names.append(name)
                shape = tuple(alloc.tensor_shape)
                dtype = mybir.dt.np(alloc.dtype)
                out_avals.append(jax.core.ShapedArray(shape, dtype))
        self.in_names = list(in_names)
        self.out_names = out_names
        self.out_avals = out_avals
        n_params = len(in_names)
        all_in_names = in_names + out_names
        if partition_name is not None:
            all_in_names.append(partition_name)

        def _body(*args):
            operands = list(args)
            if partition_name is not None:
                operands.append(partition_id_tensor())
            outs = bass2jax._bass_exec_p.bind(
                *operands,
                out_avals=tuple(out_avals),
                in_names=tuple(all_in_names),
                out_names=tuple(out_names),
                lowering_input_output_aliases=(),
                sim_require_finite=True,
                sim_require_nnan=True,
                nc=nc,
            )
            return tuple(outs)

        devices = jax.devices()[:n_cores]
        self.mesh = Mesh(np.asarray(devices), ("core",))
        in_specs = (PartitionSpec("core"),) * (n_params + len(out_names))
        out_specs = (PartitionSpec("core"),) * len(out_names)
        donate = tuple(range(n_params, n_params + len(out_names)))
        self.jitted = jax.jit(
            shard_map(_body, mesh=self.mesh, in_specs=in_specs,
                      out_specs=out_specs, check_rep=False),
            donate_argnums=donate,
            keep_unused=True,
        )
        self.sharding = jax.sharding.NamedSharding(self.mesh, PartitionSpec("core"))
        # on-device zero allocator for the donated output buffers
        zero_shapes = [(n_cores * av.shape[0], *av.shape[1:]) for av in out_avals]
        zero_dtypes = [av.dtype for av in out_avals]

        def _mk_zeros():
            import jax.numpy as jnp
            return tuple(jnp.zeros(s, d) for s, d in zip(zero_shapes, zero_dtypes))

        self._mk_zeros = jax.jit(_mk_zeros, out_shardings=(self.sharding,) * len(out_avals))
        # Recycled donation buffers: the kernel overwrites every element of
        # each ExternalOutput, so the previous call's outputs can serve as
        # the next call's donated output operands. This keeps run_staged at
        # a single dispatch+block cycle (the per-execute relay round trip
        # dominates wall time; a separate zeros allocation would double it).
        self._outbufs = None

    def concat_inputs(self, in_maps):
        assert len(in_maps) == self.n_cores
        arrs = [
            np.concatenate([np.asarray(in_maps[c][n]) for c in range(self.n_cores)], axis=0)
            for n in self.in_names
        ]
        zeros = [
            np.zeros((self.n_cores * av.shape[0], *av.shape[1:]), av.dtype)
            for av in self.out_avals
        ]
        return arrs, zeros

    def stage(self, in_maps):
        arrs, _ = self.concat_inputs(in_maps)
        staged = [jax.device_put(a, self.sharding) for a in arrs]
        if self._outbufs is None:
            self._outbufs = self._mk_zeros()
        jax.block_until_ready(staged)
        jax.block_until_ready(self._outbufs)
        return staged

    def run_staged(self, staged):
        bufs = self._outbufs
        self._outbufs = None
        if bufs is None:
            bufs = self._mk_zeros()
            jax.block_until_ready(bufs)
        outs = self.jitted(*staged, *bufs)
        jax.block_until_ready(outs)
        self._outbufs = outs
        return outs

    def __call__(self, in_maps):
        staged = self.stage(in_maps)
        outs = self.run_staged(staged)
        res = []
        for c in range(self.n_cores):
            res.append({
                name: np.asarray(outs[i]).reshape(self.n_cores, *self.out_avals[i].shape)[c]
                for i, name in enumerate(self.out_names)
            })
        return res


_NC_CACHE: dict = {}


def _get_runner():
    if "runner" not in _NC_CACHE:
        _NC_CACHE["runner"] = SpmdRunner(_build(), N_CORES)
    return _NC_CACHE["runner"]


def _host_inputs(x, Wq_sem, Wk_sem, Wq_geo, Wk_geo, Wv, Wo):
    bf16 = ml_dtypes.bfloat16
    # RoPE tables: cos replicated on both 32-row geo half-ranges; sin with
    # the rotate-half sign baked in (-sin on rows 64:96, +sin on 96:128).
    inv_freq = 1.0 / (ROPE_BASE ** (np.arange(0, 64, 2, dtype=np.float32) / 64.0))
    t = np.arange(S, dtype=np.float32)
    freqs = np.outer(t, inv_freq)  # [S, 32]
    cosT = np.cos(freqs).T.astype(np.float32)  # [32, S]
    sinT = np.sin(freqs).T.astype(np.float32)
    cs = np.zeros((128, 2 * S), np.float32)
    cs[64:96, :S] = cosT
    cs[96:128, :S] = cosT
    cs[64:96, S:] = -sinT
    cs[96:128, S:] = sinT

    # causal triangle for diagonal blocks: mask[kl, ql] = ql >= kl
    kl = np.arange(128)
    mask = (kl[None, :] >= kl[:, None]).astype(bf16)

    ones = np.ones((128, 128), np.float32)
    onesb = np.ones((128, 128), bf16)

    in_maps = []
    for c in range(N_CORES):
        b, g = divmod(c, 4)
        blocks_q, blocks_k = [], []
        for j in range(HPC):
            h = g * HPC + j
            r64 = slice(h * 64, (h + 1) * 64)
            blocks_q.append(np.concatenate([Wq_sem[r64], Wq_geo[r64]], axis=0) * SCALE)
            blocks_k.append(np.concatenate([Wk_sem[r64], Wk_geo[r64]], axis=0))
        wqk = np.ascontiguousarray(np.concatenate(blocks_q + blocks_k, axis=0).T)
        hv = slice(g * HPC * DH, (g + 1) * HPC * DH)
        wv = np.ascontiguousarray(Wv[hv].T)
        wo = np.ascontiguousarray(Wo[:, hv].T)
        xT = np.ascontiguousarray(x[b].T)
        in_maps.append({
            "xT": xT.astype(bf16),
            "wqk": wqk.astype(bf16),
            "wv": wv.astype(bf16),
            "wo": wo.astype(bf16),
            "cs": cs,
            "mask": mask,
            "ones": ones,
            "onesb": onesb,
        })
    return in_maps


def kernel(x, Wq_sem, Wk_sem, Wq_geo, Wk_geo, Wv, Wo):
    in_maps = _host_inputs(np.asarray(x), np.asarray(Wq_sem), np.asarray(Wk_sem),
                           np.asarray(Wq_geo), np.asarray(Wk_geo),
                           np.asarray(Wv), np.asarray(Wo))
    res = _get_runner()(in_maps)
    y = np.empty((B, S, D), np.float32)
    for b in range(B):
        y[b] = sum(res[b * 4 + g]["yp"] for g in range(4))
    return y


# revision 20
# speedup vs baseline: 1.0169x; 1.0074x over previous
"""DecoupledBottleneckAttention on 8 trn2 NeuronCores.

Sharding: core c -> batch b=c//4, head-group g=c%4 (4 heads/core).
Each core computes q/k/v projections for its heads, causal attention,
and a partial out-projection; the host sums the 4 partials per batch.

v2 layout:
- Single pass over x per 512-column chunk: qk(ob0-3) -> v -> qk(ob4-7)
  matmul passes share the chunk's 16 resident xT tiles, rotating 4+4
  PSUM banks so consecutive passes never wait on bank drains.
- x and all weights are bf16 (fp32 PSUM accumulation); scores and the
  softmax stay fp32; exp probabilities and v are bf16.
- RoPE as 3 DVE ops on [64,512] with the sin sign baked into the table.
- Causal diagonal blocks use variable-width rhs (512/384/256/128) and
  a single [128,128] triangle mask-mul instead of full-width masking.
- Attention (C) and out-projection (D) interleave chunk-wise:
  C0 C1 D0 C2 D1 C3 D2 D3, so D's matmuls hide C's softmax tails.
- exp() skips max-subtraction: logits are bounded (~|6|) by the fixed
  input scale. Denominators come from a ones-column matmul.
"""

import json
from contextlib import ExitStack

import numpy as np
import ml_dtypes

import jax
import concourse.bass as bass
import concourse.mybir as mybir
from concourse.tile import TileContext
from concourse import bass2jax
from concourse.bass2jax import Mesh, PartitionSpec, shard_map, partition_id_tensor

F32 = mybir.dt.float32
F32R = mybir.dt.float32r
BF16 = mybir.dt.bfloat16

B, S, D = 2, 2048, 2048
H = 16
HPC = 4  # heads per core
N_CORES = 8
DH = 128  # per-head q/k/v dim (64 sem + 64 geo; v 128)
ROPE_BASE = 10000.0
SCALE = 1.0 / np.sqrt(128.0)

NSC = S // 512  # 4 s-chunks of 512
NDT = D // 128  # 16 contraction tiles
NST = S // 128  # 16 s-tiles of 128

ACT_COPY = mybir.ActivationFunctionType.Copy
ACT_EXP = mybir.ActivationFunctionType.Exp


def _split_multi_waits(bir: dict) -> dict:
    """walrus here rejects >1 sync waits per instruction; split extras
    into single-wait Drains inserted just before, on the same engine."""
    for fn in bir.get("functions", []):
        for blk in fn.get("blocks", []):
            new_insts = []
            for ins in blk.get("instructions", []):
                si = ins.get("sync_info") or {}
                waits = si.get("on_wait") or []
                if len(waits) > 1:
                    for i, w in enumerate(waits[:-1]):
                        new_insts.append(
                            {
                                "debug": ins.get("debug", 0),
                                "engine": ins["engine"],
                                "ins": [],
                                "name": f"{ins['name']}-w{i}",
                                "opcode": "Drain",
                                "outs": [],
                                "sync_info": {"on_update": [], "on_wait": [w]},
                            }
                        )
                    si["on_wait"] = [waits[-1]]
                new_insts.append(ins)
            blk["instructions"] = new_insts
    return bir


class _PatchedBass(bass.Bass):
    def to_json_bytes(self) -> bytes:
        return json.dumps(_split_multi_waits(json.loads(super().to_json_bytes()))).encode()


def _rd(ap):
    """Bitcast a DRAM-side AP to f32r for DMAs into f32r SBUF tiles."""
    return ap.bitcast(F32R)


def _build():
    nc = _PatchedBass("TRN2", target_bir_lowering=False, debug=False, num_devices=N_CORES)

    xT_d = nc.dram_tensor("xT", [D, S], BF16, kind="ExternalInput")
    wqk_d = nc.dram_tensor("wqk", [D, 8 * 128], BF16, kind="ExternalInput")
    wv_d = nc.dram_tensor("wv", [D, HPC * DH], BF16, kind="ExternalInput")
    wo_d = nc.dram_tensor("wo", [HPC * DH, D], BF16, kind="ExternalInput")
    # rows 64:128 hold cos (cols 0:S) and sign-baked sin (cols S:2S:
    # rows 64:96 = -sin, rows 96:128 = +sin); rows 0:64 unused.
    cs_d = nc.dram_tensor("cs", [128, 2 * S], F32, kind="ExternalInput")
    mask_d = nc.dram_tensor("mask", [128, 128], BF16, kind="ExternalInput")
    ones_d = nc.dram_tensor("ones", [128, 128], F32, kind="ExternalInput")
    onesb_d = nc.dram_tensor("onesb", [128, 128], BF16, kind="ExternalInput")
    yp_d = nc.dram_tensor("yp", [S, D], F32, kind="ExternalOutput")

    with TileContext(nc) as tc, ExitStack() as ctx, \
         nc.allow_low_precision(reason="float32r tiles are 4-byte fp32 at rest"):
        pers = ctx.enter_context(tc.tile_pool(name="pers", bufs=1))
        # qkT[0..3] = per-head qT [128 dims, S]; qkT[4..7] = kT
        qkT = [pers.tile([128, S], F32R, name=f"qkT{i}", tag=f"qkT{i}") for i in range(8)]
        v_sb = [pers.tile([128, HPC * DH], BF16, name=f"v{st}", tag=f"v{st}")
                for st in range(NST)]
        cs_sb = pers.tile([128, 2 * S], F32, name="cs_sb", tag="cs_sb")
        ones_sb = pers.tile([128, 128], F32R, name="ones_sb", tag="ones_sb")
        onesb_sb = pers.tile([128, 128], BF16, name="onesb_sb", tag="onesb_sb")
        mask_sb = pers.tile([128, 128], BF16, name="mask_sb", tag="mask_sb")

        # ------------- Phase A+B: q/k/v projections, one x pass --------
        with tc.tile_pool(name="wqk", bufs=1) as wqk_pool, \
             tc.tile_pool(name="wv", bufs=1) as wv_pool, \
             tc.tile_pool(name="xt", bufs=2) as xt_pool, \
             tc.tile_pool(name="rope", bufs=4) as rope_pool, \
             tc.tile_pool(name="psQK", bufs=1, space="PSUM") as psQK, \
             tc.tile_pool(name="psV", bufs=1, space="PSUM") as psV:
            wqk_sb = [wqk_pool.tile([128, 8 * 128], BF16, name=f"wqk{dt}", tag=f"wqk{dt}")
                      for dt in range(NDT)]
            wv_sb = [wv_pool.tile([128, HPC * DH], BF16, name=f"wv{dt}", tag=f"wv{dt}")
                     for dt in range(NDT)]

            def xt_tiles(sc):
                tiles = [xt_pool.tile([128, 512], BF16, name="xt_t", tag=f"xt{dt}")
                         for dt in range(NDT)]
                for dt in range(NDT):
                    nc.sync.dma_start(
                        out=tiles[dt],
                        in_=xT_d[dt * 128:(dt + 1) * 128, sc * 512:(sc + 1) * 512])
                return tiles

            # chunk-0 stream: weights interleaved with x tiles so the first
            # matmul's operands arrive first; bulk tables after.
            xt_cur = [xt_pool.tile([128, 512], BF16, name="xt_t", tag=f"xt{dt}")
                      for dt in range(NDT)]
            for dt in range(NDT):
                nc.sync.dma_start(out=wqk_sb[dt], in_=wqk_d[dt * 128:(dt + 1) * 128, :])
                nc.sync.dma_start(
                    out=xt_cur[dt], in_=xT_d[dt * 128:(dt + 1) * 128, 0:512])
            for dt in range(NDT):
                nc.sync.dma_start(out=wv_sb[dt], in_=wv_d[dt * 128:(dt + 1) * 128, :])
            nc.sync.dma_start(out=cs_sb, in_=cs_d[:, :])
            nc.sync.dma_start(out=ones_sb, in_=_rd(ones_d[:, :]))
            nc.sync.dma_start(out=onesb_sb, in_=onesb_d[:, :])
            nc.sync.dma_start(out=mask_sb, in_=mask_d[:, :])

            Am, Bm = slice(64, 96), slice(96, 128)

            def rope_out(ob, cols):
                """PSUM qk tile -> qkT[ob] with RoPE on the geo rows."""
                ps = ps_qk[ob % 4]
                dst = qkT[ob]
                nc.scalar.activation(dst[0:64, cols], ps[0:64, :], ACT_COPY)
                stage = rope_pool.tile([128, 512], F32, name="ropest", tag="ropest")
                sw = rope_pool.tile([128, 512], F32, name="ropesw", tag="ropesw")
                # stage copy on DVE: halves the ACT backlog that would queue
                # the attention phase's first exp() behind rope drains.
                nc.vector.tensor_scalar_mul(stage[64:128, :], ps[64:128, :], 1.0)
                nc.sync.dma_start(out=sw[Am, :], in_=stage[Bm, :])  # x2 -> A rows
                nc.sync.dma_start(out=sw[Bm, :], in_=stage[Am, :])  # x1 -> B rows
                csc = cs_sb[64:128, sc * 512:(sc + 1) * 512]
                sns = cs_sb[64:128, S + sc * 512:S + (sc + 1) * 512]
                g = slice(64, 128)
                tp = rope_pool.tile([128, 512], F32, name="ropetp", tag="ropetp")
                nc.vector.tensor_mul(tp[g, :], stage[g, :], csc)     # [x1;x2]*cos
                nc.vector.tensor_mul(sw[g, :], sw[g, :], sns)        # [x2;x1]*(-/+sin)
                nc.vector.tensor_add(dst[g, cols], tp[g, :], sw[g, :])

            for sc in range(NSC):
                cols = slice(sc * 512, (sc + 1) * 512)
                if sc + 1 < NSC:
                    xt_next = xt_tiles(sc + 1)
                else:
                    xt_next = None
                # pass 1: q/k heads 0-3
                ps_qk = [psQK.tile([128, 512], F32, name=f"psqk{i}", tag=f"psqk{i}")
                         for i in range(4)]
                for dt in range(NDT):
                    for ob in range(4):
                        nc.tensor.matmul(
                            ps_qk[ob],
                            lhsT=(wqk_sb[dt][:, ob * 128:(ob + 1) * 128]),
                            rhs=(xt_cur[dt]),
                            start=(dt == 0), stop=(dt == NDT - 1),
                        )
                for ob in range(4):
                    rope_out(ob, cols)
                # passes 2+3: v projection (natural [s, head-dim] layout)
                # and q/k heads 4-7. Normal chunks run v between the two qk
                # passes so each pass's PSUM banks drain during the next.
                # The LAST chunk runs v last: the attention phase's entry
                # barrier then waits only on v's short ACT-copy tail instead
                # of the longer rope DVE chains.
                def v_pass():
                    psv = [psV.tile([128, HPC * DH], F32, name=f"psv{st}", tag=f"psv{st}")
                           for st in range(4)]
                    for dt in range(NDT):
                        for st in range(4):
                            nc.tensor.matmul(
                                psv[st],
                                lhsT=(xt_cur[dt][:, st * 128:(st + 1) * 128]),
                                rhs=(wv_sb[dt]),
                                start=(dt == 0), stop=(dt == NDT - 1),
                            )
                    for st in range(4):
                        nc.scalar.activation(v_sb[sc * 4 + st], psv[st], ACT_COPY)

                def qkb_pass():
                    qb = [psQK.tile([128, 512], F32, name=f"psqk{i}", tag=f"psqk{i}")
                          for i in range(4)]
                    for dt in range(NDT):
                        for ob in range(4):
                            nc.tensor.matmul(
                                qb[ob],
                                lhsT=(wqk_sb[dt][:, (ob + 4) * 128:(ob + 5) * 128]),
                                rhs=(xt_cur[dt]),
                                start=(dt == 0), stop=(dt == NDT - 1),
                            )
                    return qb

                if sc == NSC - 1:
                    ps_qk = qkb_pass()
                    for ob in range(4):
                        rope_out(ob + 4, cols)
                    v_pass()
                else:
                    v_pass()
                    ps_qk = qkb_pass()
                    for ob in range(4):
                        rope_out(ob + 4, cols)
                if xt_next is not None:
                    xt_cur = xt_next

        # ------------- Phase C+D: attention + out-projection -----------
        with tc.tile_pool(name="wo", bufs=1) as wo_pool, \
             tc.tile_pool(name="outT", bufs=1) as outT_pool, \
             tc.tile_pool(name="attn", bufs=5) as attn_pool, \
             tc.tile_pool(name="lrec", bufs=2) as lrec_pool, \
             tc.tile_pool(name="ysb", bufs=5) as y_pool, \
             tc.tile_pool(name="psL", bufs=1, space="PSUM") as psL, \
             tc.tile_pool(name="psR", bufs=1, space="PSUM") as psR, \
             tc.tile_pool(name="psD", bufs=2, space="PSUM") as psD, \
             tc.tile_pool(name="psST", bufs=2, space="PSUM") as psST, \
             tc.tile_pool(name="psOut", bufs=2, space="PSUM") as psOut:
            wo_sb = [wo_pool.tile([128, D], BF16, name=f"wo{j}", tag=f"wo{j}")
                     for j in range(HPC)]
            for j in range(HPC):
                nc.sync.dma_start(out=wo_sb[j], in_=wo_d[j * 128:(j + 1) * 128, :])
            outT = [outT_pool.tile([128, S], BF16, name=f"outT{j}", tag=f"outT{j}")
                    for j in range(HPC)]

            def attn_chunk(qc):
                qcols = slice(qc * 512, (qc + 1) * 512)
                kmax = qc * 4 + 4
                for j in range(HPC):
                    outp = psOut.tile([128, 512], F32, name="outp", tag="outp")
                    lp = psL.tile([1, 512], F32, name="lp", tag="lp")
                    for kj in range(kmax):
                        d = kj - qc * 4
                        qs = 0 if d < 0 else d * 128
                        w = 512 - qs
                        st_ps = psST.tile([128, 512], F32, name="st_ps", tag="st_ps")
                        nc.tensor.matmul(
                            st_ps[:, 0:w],
                            lhsT=(qkT[4 + j][:, kj * 128:(kj + 1) * 128]),
                            rhs=(qkT[j][:, qc * 512 + qs:(qc + 1) * 512]),
                            start=True, stop=True,
                        )
                        p_sb = attn_pool.tile([128, 512], BF16, name="p_sb", tag="p_sb")
                        nc.scalar.activation(p_sb[:, qs:512], st_ps[:, 0:w], ACT_EXP)
                        if d >= 0:
                            nc.vector.tensor_mul(
                                p_sb[:, qs:qs + 128], p_sb[:, qs:qs + 128], mask_sb)
                        nc.tensor.matmul(
                            outp[:, qs:512],
                            lhsT=(v_sb[kj][:, j * DH:(j + 1) * DH]),
                            rhs=(p_sb[:, qs:512]),
                            start=(kj == 0), stop=(kj == kmax - 1),
                        )
                        nc.tensor.matmul(
                            lp[:, qs:512],
                            lhsT=(onesb_sb[:, 0:1]),
                            rhs=(p_sb[:, qs:512]),
                            start=(kj == 0), stop=(kj == kmax - 1),
                        )
                    # DVE reads the denominator PSUM directly (its one
                    # allowed PSUM operand) - no ACT staging copy needed
                    r_sb = lrec_pool.tile([1, 512], F32R, name="r_sb", tag="r_sb")
                    nc.vector.reciprocal(r_sb, lp)
                    rp = psR.tile([128, 512], F32, name="rp", tag="rp")
                    nc.tensor.matmul(rp, lhsT=(ones_sb[0:1, :]),
                                     rhs=(r_sb), start=True, stop=True)
                    # DVE may read only one PSUM operand: stage rp in SBUF
                    rbc = lrec_pool.tile([128, 512], F32, name="rbc", tag="rbc")
                    nc.scalar.activation(rbc, rp, ACT_COPY)
                    nc.vector.tensor_mul(outT[j][:, qcols], outp, rbc)

            def outproj_chunk(qc):
                for st4 in range(4):
                    st = qc * 4 + st4
                    for mc in range(NSC):
                        yp_ps = psD.tile([128, 512], F32, name="yp_ps", tag="yp_ps")
                        for j in range(HPC):
                            nc.tensor.matmul(
                                yp_ps,
                                lhsT=(outT[j][:, st * 128:(st + 1) * 128]),
                                rhs=(wo_sb[j][:, mc * 512:(mc + 1) * 512]),
                                start=(j == 0), stop=(j == HPC - 1),
                            )
                        y_sb = y_pool.tile([128, 512], F32, name="y_sb", tag="y_sb")
                        nc.vector.tensor_scalar_mul(y_sb, yp_ps, 1.0)
                        nc.sync.dma_start(
                            out=yp_d[st * 128:(st + 1) * 128, mc * 512:(mc + 1) * 512],
                            in_=y_sb)

            # software pipeline: D(qc) runs one chunk behind C(qc);
            # chunk-0 attention gets scheduler priority so its exp() ops
            # are not queued behind the projection phase's last rope drains
            with tc.high_priority():
                attn_chunk(0)
            attn_chunk(1)
            outproj_chunk(0)
            attn_chunk(2)
            outproj_chunk(1)
            attn_chunk(3)
            outproj_chunk(2)
            outproj_chunk(3)
    return nc


class SpmdRunner:
    def __init__(self, nc, n_cores: int):
        bass2jax.install_neuronx_cc_hook()
        self.nc = nc
        self.n_cores = n_cores
        partition_name = nc.partition_id_tensor.name if nc.partition_id_tensor else None

        in_names, out_names, out_avals = [], [], []
        for alloc in nc.m.functions[0].allocations:
            if not isinstance(alloc, mybir.MemoryLocationSet):
                continue
            name = alloc.memorylocations[0].name
            if alloc.kind == "ExternalInput":
                if name != partition_name:
                    in_names.append(name)
            elif alloc.kind == "ExternalOutput":
                out_names.append(name)
                shape = tuple(alloc.tensor_shape)
                dtype = mybir.dt.np(alloc.dtype)
                out_avals.append(jax.core.ShapedArray(shape, dtype))
        self.in_names = list(in_names)
        self.out_names = out_names
        self.out_avals = out_avals
        n_params = len(in_names)
        all_in_names = in_names + out_names
        if partition_name is not None:
            all_in_names.append(partition_name)

        def _body(*args):
            operands = list(args)
            if partition_name is not None:
                operands.append(partition_id_tensor())
            outs = bass2jax._bass_exec_p.bind(
                *operands,
                out_avals=tuple(out_avals),
                in_names=tuple(all_in_names),
                out_names=tuple(out_names),
                lowering_input_output_aliases=(),
                sim_require_finite=True,
                sim_require_nnan=True,
                nc=nc,
            )
            return tuple(outs)

        devices = jax.devices()[:n_cores]
        self.mesh = Mesh(np.asarray(devices), ("core",))
        in_specs = (PartitionSpec("core"),) * (n_params + len(out_names))
        out_specs = (PartitionSpec("core"),) * len(out_names)
        donate = tuple(range(n_params, n_params + len(out_names)))
        self.jitted = jax.jit(
            shard_map(_body, mesh=self.mesh, in_specs=in_specs,
                      out_specs=out_specs, check_rep=False),
            donate_argnums=donate,
            keep_unused=True,
        )
        self.sharding = jax.sharding.NamedSharding(self.mesh, PartitionSpec("core"))
        # on-device zero allocator for the donated output buffers
        zero_shapes = [(n_cores * av.shape[0], *av.shape[1:]) for av in out_avals]
        zero_dtypes = [av.dtype for av in out_avals]

        def _mk_zeros():
            import jax.numpy as jnp
            return tuple(jnp.zeros(s, d) for s, d in zip(zero_shapes, zero_dtypes))

        self._mk_zeros = jax.jit(_mk_zeros, out_shardings=(self.sharding,) * len(out_avals))
        # Recycled donation buffers: the kernel overwrites every element of
        # each ExternalOutput, so the previous call's outputs can serve as
        # the next call's donated output operands. This keeps run_staged at
        # a single dispatch+block cycle (the per-execute relay round trip
        # dominates wall time; a separate zeros allocation would double it).
        self._outbufs = None

    def concat_inputs(self, in_maps):
        assert len(in_maps) == self.n_cores
        arrs = [
            np.concatenate([np.asarray(in_maps[c][n]) for c in range(self.n_cores)], axis=0)
            for n in self.in_names
        ]
        zeros = [
            np.zeros((self.n_cores * av.shape[0], *av.shape[1:]), av.dtype)
            for av in self.out_avals
        ]
        return arrs, zeros

    def stage(self, in_maps):
        arrs, _ = self.concat_inputs(in_maps)
        staged = [jax.device_put(a, self.sharding) for a in arrs]
        if self._outbufs is None:
            self._outbufs = self._mk_zeros()
        jax.block_until_ready(staged)
        jax.block_until_ready(self._outbufs)
        return staged

    def run_staged(self, staged):
        bufs = self._outbufs
        self._outbufs = None
        if bufs is None:
            bufs = self._mk_zeros()
            jax.block_until_ready(bufs)
        outs = self.jitted(*staged, *bufs)
        jax.block_until_ready(outs)
        self._outbufs = outs
        return outs

    def __call__(self, in_maps):
        staged = self.stage(in_maps)
        outs = self.run_staged(staged)
        res = []
        for c in range(self.n_cores):
            res.append({
                name: np.asarray(outs[i]).reshape(self.n_cores, *self.out_avals[i].shape)[c]
                for i, name in enumerate(self.out_names)
            })
        return res


_NC_CACHE: dict = {}


def _get_runner():
    if "runner" not in _NC_CACHE:
        _NC_CACHE["runner"] = SpmdRunner(_build(), N_CORES)
    return _NC_CACHE["runner"]


def _host_inputs(x, Wq_sem, Wk_sem, Wq_geo, Wk_geo, Wv, Wo):
    bf16 = ml_dtypes.bfloat16
    # RoPE tables: cos replicated on both 32-row geo half-ranges; sin with
    # the rotate-half sign baked in (-sin on rows 64:96, +sin on 96:128).
    inv_freq = 1.0 / (ROPE_BASE ** (np.arange(0, 64, 2, dtype=np.float32) / 64.0))
    t = np.arange(S, dtype=np.float32)
    freqs = np.outer(t, inv_freq)  # [S, 32]
    cosT = np.cos(freqs).T.astype(np.float32)  # [32, S]
    sinT = np.sin(freqs).T.astype(np.float32)
    cs = np.zeros((128, 2 * S), np.float32)
    cs[64:96, :S] = cosT
    cs[96:128, :S] = cosT
    cs[64:96, S:] = -sinT
    cs[96:128, S:] = sinT

    # causal triangle for diagonal blocks: mask[kl, ql] = ql >= kl
    kl = np.arange(128)
    mask = (kl[None, :] >= kl[:, None]).astype(bf16)

    ones = np.ones((128, 128), np.float32)
    onesb = np.ones((128, 128), bf16)

    in_maps = []
    for c in range(N_CORES):
        b, g = divmod(c, 4)
        blocks_q, blocks_k = [], []
        for j in range(HPC):
            h = g * HPC + j
            r64 = slice(h * 64, (h + 1) * 64)
            blocks_q.append(np.concatenate([Wq_sem[r64], Wq_geo[r64]], axis=0) * SCALE)
            blocks_k.append(np.concatenate([Wk_sem[r64], Wk_geo[r64]], axis=0))
        wqk = np.ascontiguousarray(np.concatenate(blocks_q + blocks_k, axis=0).T)
        hv = slice(g * HPC * DH, (g + 1) * HPC * DH)
        wv = np.ascontiguousarray(Wv[hv].T)
        wo = np.ascontiguousarray(Wo[:, hv].T)
        xT = np.ascontiguousarray(x[b].T)
        in_maps.append({
            "xT": xT.astype(bf16),
            "wqk": wqk.astype(bf16),
            "wv": wv.astype(bf16),
            "wo": wo.astype(bf16),
            "cs": cs,
            "mask": mask,
            "ones": ones,
            "onesb": onesb,
        })
    return in_maps


def kernel(x, Wq_sem, Wk_sem, Wq_geo, Wk_geo, Wv, Wo):
    in_maps = _host_inputs(np.asarray(x), np.asarray(Wq_sem), np.asarray(Wk_sem),
                           np.asarray(Wq_geo), np.asarray(Wk_geo),
                           np.asarray(Wv), np.asarray(Wo))
    res = _get_runner()(in_maps)
    y = np.empty((B, S, D), np.float32)
    for b in range(B):
        y[b] = sum(res[b * 4 + g]["yp"] for g in range(4))
    return y
